# revision 26
# baseline (speedup 1.0000x reference)
"""Trainium2 Bass kernel for a dense transformer block (pre-LN, 6-head causal
attention, 4x FFN) over x:(128,256,384) f32.

Strategy: pure data-parallel over batch across 8 NeuronCores (16 sequences per
core). Per-core Tile kernel computes the whole block per sequence:
  LN1 -> QKV (bf16 matmuls, LN scale/shift folded into weights on host)
  -> causal softmax (no max-subtract; scores are tiny by construction)
  -> P@V via PE transposes -> proj + residual -> LN2 -> FFN (relu) + residual.
All matmul operands are bf16 (fp32 PSUM accumulation); the residual stream
stays fp32 end-to-end. ACT stays on one function table (ln/exp/copy/relu);
rsqrt is computed as exp(-0.5*ln(var+eps)).
"""

import os
import sys

import numpy as np

try:
    import concourse.bass as bass  # noqa: F401
except ImportError:
    sys.path.insert(0, "/opt/trn_rl_repo")

import ml_dtypes
from contextlib import ExitStack

import concourse.bass as bass
import concourse.tile as tile
from concourse import bacc, mybir
from concourse.bass_utils import run_bass_kernel_spmd

BF16 = ml_dtypes.bfloat16
FP8 = ml_dtypes.float8_e4m3

N_CORES = 8
B, T, C = 128, 256, 384
H, DH = 6, 64
F = 4 * C  # 1536
BPC = B // N_CORES  # sequences per core
NT = T // 128  # 2 t-tiles
NC_ = C // 128  # 3 c-chunks
NF = F // 128  # 12 f-chunks
LN_EPS = 1e-5

AF = mybir.ActivationFunctionType
ALU = mybir.AluOpType
F32 = mybir.dt.float32
BF = mybir.dt.bfloat16
F8 = mybir.dt.float8e4
DR = mybir.MatmulPerfMode.DoubleRow

# packed-weight blob layouts: name -> (element offset, free-dim shape)
F8_SEGS = {
    "wp": (0, (NC_, C)),
    "w1": (NC_ * C, (NC_, F)),
    "w2": (NC_ * C + NC_ * F, (NF, C)),
    "wq": (NC_ * C + NC_ * F + NF * C, (NC_, C)),
    "wk": (2 * NC_ * C + NC_ * F + NF * C, (NC_, C)),
    "wv": (3 * NC_ * C + NC_ * F + NF * C, (NC_, C)),
}
F8_MID = NC_ * C + NC_ * F + NF * C  # wq/wk/wv tail deferred one round
F8_TOT = 4 * NC_ * C + NC_ * F + NF * C
BF_SEGS = {
    "ident": (0, (128,)),
    "ones64": (128, (64,)),
    "trineg": (192, (128,)),
    "cmaskT": (320, (C,)),
}
BF_TOT = 320 + C
F32_SEGS = {
    "bq": (0, (NC_,)),
    "bk": (NC_, (NC_,)),
    "b1e": (2 * NC_, (NF,)),
}
F32_TOT = 2 * NC_ + NF

SB_SHORT = int(os.environ.get("KERNEL_SBUFS_SHORT", "0")) or None
FINE_YIELD = int(os.environ.get("KERNEL_FINE_YIELD", "0"))
COARSE_YIELD = int(os.environ.get("KERNEL_COARSE_YIELD", "0"))

_PROGRAM_CACHE = {}
LAST_EXEC_NS = None
LAST_RESULTS = None
PROFILE = bool(int(os.environ.get("KERNEL_PROFILE", "0")))
DMA_T_XN = bool(int(os.environ.get("KERNEL_DMA_T_XN", "0")))
ATPACK = bool(int(os.environ.get("KERNEL_ATPACK", "1")))
DMA_T_P = bool(int(os.environ.get("KERNEL_DMA_T_P", "0")))
TRACE_DIR = os.environ.get("KERNEL_TRACE_DIR") or None


def _bcast_h(ap, n):
    """Insert a stride-0 dim of size n after the partition dim of a 2D AP."""
    return bass.AP(
        tensor=ap.tensor, offset=ap.offset, ap=[ap.ap[0], [0, n], ap.ap[1]]
    )


def _emit_seq(nc, tc, pools, cst, b, x_d, out_d, flags):
    """Emit IR for one sequence b."""
    (wpool, xpool, apool, spool, psA, psB, psS, psT) = pools

    # ---- load x (T-major: partition = t%128) ----
    x_sb = xpool.tile([128, NT, C], F32, tag="x", name="x_sb")
    nc.sync.dma_start(x_sb[:], x_d[b].rearrange("(tt p) c -> p tt c", p=128))

    def layer_norm_to_bf16(src_sb, tag):
        """bn_stats/aggr per t-tile; returns (x - mu) * rstd as bf16.
        rstd = exp(-0.5 * ln(var + eps)) keeps ACT on the ln/exp table."""
        st = spool.tile([128, NT, 6], F32, tag=f"st{tag}", name="st")
        mv = spool.tile([128, NT, 2], F32, tag=f"mv{tag}", name="mv")
        for tt in range(NT):
            nc.vector.bn_stats(st[:, tt], src_sb[:, tt])
            nc.vector.bn_aggr(mv[:, tt], st[:, tt])
        rstd = spool.tile([128, NT], F32, tag=f"rstd{tag}", name="rstd")
        if int(os.environ.get("KERNEL_RSTD_POOL", "0")):
            # Newton rsqrt on the (idle) Pool engine; var+eps is within
            # [0.5, 2] for LN of ~unit-variance rows, so y0 = 1.5 - 0.5v
            # converges to <0.1% in 3 iterations.
            v = mv[:, :, 1]
            tmp = spool.tile([128, NT], F32, tag=f"nt{tag}", name="nt")
            nc.gpsimd.tensor_scalar(
                out=rstd[:], in0=v, scalar1=-0.5, scalar2=1.5,
                op0=ALU.mult, op1=ALU.add,
            )
            for _ in range(3):
                nc.gpsimd.tensor_mul(tmp[:], rstd[:], rstd[:])
                nc.gpsimd.tensor_mul(tmp[:], tmp[:], v)
                nc.gpsimd.tensor_scalar(
                    out=tmp[:], in0=tmp[:], scalar1=-0.5, scalar2=1.5,
                    op0=ALU.mult, op1=ALU.add,
                )
                nc.gpsimd.tensor_mul(rstd[:], rstd[:], tmp[:])
        else:
            lnv = spool.tile([128, NT], F32, tag=f"lnv{tag}", name="lnv")
            nc.scalar.activation(lnv[:], mv[:, :, 1], AF.Ln, bias=cst["eps"][:, 0:1])
            nc.scalar.activation(rstd[:], lnv[:], AF.Exp, scale=-0.5)
        xn = xpool.tile([128, NT, C], BF, tag=f"xn{tag}", name="xn")
        for tt in range(NT):
            nc.gpsimd.tensor_scalar(
                out=xn[:, tt],
                in0=src_sb[:, tt],
                scalar1=mv[:, tt, 0:1],
                scalar2=rstd[:, tt : tt + 1],
                op0=ALU.subtract,
                op1=ALU.mult,
            )
        return xn

    def transpose_to_cmajor(xn, tag):
        """[128, NT, C] bf16 -> [128, NC_, T] fp8 via PE transposes (bf16)
        with the fp8 cast folded into the PSUM->SBUF copy."""
        xnT = xpool.tile([128, NC_, T], F8, tag=f"xnT{tag}", name="xnT")
        if DMA_T_XN:
            for cc in range(NC_):
                for tt in range(NT):
                    nc.sync.dma_start_transpose(
                        xnT[:, cc, tt * 128 : (tt + 1) * 128],
                        xn[:, tt, cc * 128 : (cc + 1) * 128],
                    )
            return xnT
        tp = psT.tile([128, NC_, T], BF, tag="tps", name="tp")
        for cc in range(NC_):
            for tt in range(NT):
                nc.tensor.transpose(
                    tp[:, cc, tt * 128 : (tt + 1) * 128],
                    xn[:, tt, cc * 128 : (cc + 1) * 128],
                    cst["ident"][:],
                )
        nc.vector.tensor_copy(xnT[:], tp[:])
        return xnT

    xn1 = layer_norm_to_bf16(x_sb, "1")
    xnT = transpose_to_cmajor(xn1, "1")
    yield

    # ---- QKV projections (C-major Q/K, T-major V) ----
    # Q and K share one tile so their dt=2 PSUM copies merge into a single
    # (strided) ACT instruction
    QKT = apool.tile([128, 2, NC_, T], BF, tag="QT", name="QKT", bufs=SB_SHORT)
    QT, KT = QKT[:, 0], QKT[:, 1]
    ps2 = psB.tile([128, 2, T], F32, tag="psB", name="ps_qk2")
    for qi, (w_sb, b_sb, b_nz) in enumerate((
        (cst["wq"], cst["bq"], flags["bq_nz"]),
        (cst["wk"], cst["bk"], flags["bk_nz"]),
    )):
        dst = QKT[:, qi]
        ps01 = psA.tile([128, 2, T], F32, tag="psA", name="ps_qk01")
        for dt in range(NC_):
            ps = ps01[:, dt] if dt < 2 else ps2[:, qi]
            nc.tensor.matmul(
                ps,
                w_sb[:, 0:2, dt * 128 : (dt + 1) * 128],
                xnT[:, 0:2],
                start=True,
                stop=False,
                perf_mode=DR,
            )
            nc.tensor.matmul(
                ps,
                w_sb[:, 2, dt * 128 : (dt + 1) * 128],
                xnT[:, 2],
                start=False,
                stop=True,
            )
        if b_nz:
            for dt in range(NC_):
                ps = ps01[:, dt] if dt < 2 else ps2[:, qi]
                nc.scalar.activation(
                    dst[:, dt], ps, AF.Identity, bias=b_sb[:, dt : dt + 1]
                )
        else:
            nc.scalar.copy(dst[:, 0:2], ps01[:])
    if not flags["bq_nz"] and not flags["bk_nz"]:
        nc.scalar.copy(QKT[:, :, 2], ps2[:])
    elif not flags["bq_nz"]:
        nc.scalar.copy(QKT[:, 0, 2], ps2[:, 0])
    elif not flags["bk_nz"]:
        nc.scalar.copy(QKT[:, 1, 2], ps2[:, 1])
    V = apool.tile([128, NT, C], BF, tag="V", name="V", bufs=SB_SHORT)
    for tt in range(NT):
        ps = psB.tile([128, C], F32, tag="psB", name="ps_v")
        nc.tensor.matmul(
            ps[:],
            xnT[:, 0:2, tt * 128 : (tt + 1) * 128],
            cst["wv"][:, 0:2],
            start=True,
            stop=False,
            perf_mode=DR,
        )
        nc.tensor.matmul(
            ps[:],
            xnT[:, 2, tt * 128 : (tt + 1) * 128],
            cst["wv"][:, 2],
            start=False,
            stop=True,
        )
        if flags["bv_nz"]:
            nc.vector.tensor_add(ps[:], ps[:], cst["bv_bc"][:])
        nc.vector.tensor_copy(V[:, tt], ps[:])
    if not COARSE_YIELD:
        yield

    # ---- attention (transposed-score form) ----
    # ET layout per head: [128, H, 384]: cols 0:256 = (s0 x t0..t1), 256:384
    # = (s1 x t1). Scores come out pre-transposed (K stationary, Q moving),
    # so P never needs a PE transpose; softmax denominators are computed by
    # ones-matmuls replicated across 64 partitions, and the normalization is
    # applied AFTER P@V (it commutes with the V contraction).
    ET = apool.tile([128, H, 384], BF, tag="E", name="ET", bufs=SB_SHORT)
    rec = spool.tile([128, NC_, T], F32, tag="rec", name="rec", bufs=SB_SHORT)
    attnT = apool.tile([128, NC_, T], F8, tag="attnT", name="attnT", bufs=SB_SHORT)
    mask_mode = os.environ.get("KERNEL_MASK", "pe")

    for h in range(H):
        dt, off = h // 2, (h % 2) * 64
        sc = psS.tile([128, 384], F32, tag="sc", name="sc")
        nc.tensor.matmul(
            sc[:, 0:256],
            KT[off : off + 64, dt, 0:128],
            QT[off : off + 64, dt, :],
            start=True,
            stop=True,
        )
        nc.tensor.matmul(
            sc[:, 256:384],
            KT[off : off + 64, dt, 128:256],
            QT[off : off + 64, dt, 128:256],
            start=True,
            stop=True,
        )
        if mask_mode == "pe":
            # add -30 to causally-masked positions so exp() flushes them to
            # ~1e-13 (constant-matrix add via identity matmul; no DVE pass)
            nc.tensor.matmul(
                sc[:, 0:128], cst["ident"][:], cst["trineg"][:],
                start=False, stop=True, skip_group_check=True,
            )
            nc.tensor.matmul(
                sc[:, 256:384], cst["ident"][:], cst["trineg"][:],
                start=False, stop=True, skip_group_check=True,
            )
        nc.scalar.activation(ET[:, h], sc[:], AF.Exp)
        if mask_mode in ("pool", "dve") and h % 2 == 1:
            # zero the causally-masked positions by multiplying the two
            # triangular blocks of this head pair (cols 0:128 and 256:384)
            # with the triu mask, broadcast from cmaskT's first 128 cols
            et0 = ET[:, h - 1]
            blocks = bass.AP(
                tensor=et0.tensor,
                offset=et0.offset,
                ap=[et0.ap[0], [384, 2], [256, 2], [1, 128]],
            )
            cm = cst["cmaskT"][:]
            tri_bc = bass.AP(
                tensor=cm.tensor,
                offset=cm.offset,
                ap=[cm.ap[0], [0, 2], [0, 2], [1, 128]],
            )
            eng = nc.gpsimd if mask_mode == "pool" else nc.vector
            eng.tensor_mul(blocks, blocks, tri_bc)
        if FINE_YIELD and h == 2:
            yield
    if COARSE_YIELD < 2:
        yield

    for pair in range(H // 2):
        # softmax denominators for heads (2*pair, 2*pair+1), replicated
        # across 64 partitions each via ones-matmul
        sm = psT.tile([128, T], F32, tag="tps", name="sums")
        for s_i in range(2):
            h = pair * 2 + s_i
            tp_kw = {} if s_i == 0 else {"tile_position": (0, 64)}
            sl = slice(s_i * 64, (s_i + 1) * 64)
            nc.tensor.matmul(
                sm[sl, 0:256], cst["ones64"][:], ET[:, h, 0:256],
                start=True, stop=False, skip_group_check=True, **tp_kw,
            )
            nc.tensor.matmul(
                sm[sl, 128:256], cst["ones64"][:], ET[:, h, 256:384],
                start=False, stop=True, skip_group_check=True, **tp_kw,
            )
        nc.vector.reciprocal(rec[:, pair], sm[:])

        at = psT.tile([128, 256], F32, tag="tps", name="atp")
        for s_i in range(2):
            h = pair * 2 + s_i
            dh = h * DH
            tp_kw = {} if s_i == 0 else {"tile_position": (0, 64)}
            sl = slice(s_i * 64, (s_i + 1) * 64)
            nc.tensor.matmul(
                at[sl, 0:256], V[:, 0, dh : dh + 64], ET[:, h, 0:256],
                start=True, stop=False, skip_group_check=True, **tp_kw,
            )
            nc.tensor.matmul(
                at[sl, 128:256], V[:, 1, dh : dh + 64], ET[:, h, 256:384],
                start=False, stop=True, skip_group_check=True, **tp_kw,
            )
        # normalize (divide by per-(head,t) denominator) + cast to fp8
        nc.vector.tensor_mul(attnT[:, pair], at[:], rec[:, pair])
        if FINE_YIELD and pair == 0:
            yield
    yield

    # ---- projection + residual ----
    x2 = xpool.tile([128, NT, C], F32, tag="x2", name="x2")
    for tt in range(NT):
        ps = psB.tile([128, C], F32, tag="psB", name="ps_proj")
        nc.tensor.matmul(
            ps[:],
            attnT[:, 0:2, tt * 128 : (tt + 1) * 128],
            cst["wp"][:, 0:2],
            start=True,
            stop=False,
            perf_mode=DR,
        )
        nc.tensor.matmul(
            ps[:],
            attnT[:, 2, tt * 128 : (tt + 1) * 128],
            cst["wp"][:, 2],
            start=False,
            stop=True,
        )
        if int(os.environ.get("KERNEL_RESID_POOL", "0")):
            ycp = spool.tile([128, NT, C], F32, tag="ycp", name="ycp")
            nc.scalar.copy(ycp[:, tt], ps[:])
            nc.gpsimd.tensor_add(x2[:, tt], x_sb[:, tt], ycp[:, tt])
        else:
            nc.vector.tensor_add(x2[:, tt], x_sb[:, tt], ps[:])
        if flags["bp_nz"]:
            nc.vector.tensor_add(x2[:, tt], x2[:, tt], cst["bp_bc"][:])

    # ---- LN2 + FFN ----
    xn2 = layer_norm_to_bf16(x2, "2")
    xn2T = transpose_to_cmajor(xn2, "2")
    yield

    zT = apool.tile([128, NF, T], F8, tag="zT", name="zT", bufs=SB_SHORT)

    def ffn1_matmuls(ps, ft):
        nc.tensor.matmul(
            ps,
            cst["w1"][:, 0:2, ft * 128 : (ft + 1) * 128],
            xn2T[:, 0:2],
            start=True,
            stop=False,
            perf_mode=DR,
        )
        nc.tensor.matmul(
            ps,
            cst["w1"][:, 2, ft * 128 : (ft + 1) * 128],
            xn2T[:, 2],
            start=False,
            stop=True,
        )

    if not flags["b1_nz"]:
        # paired f-tiles: one [128,512] psum bank, one relu per pair
        for fp in range(NF // 2):
            ps = psA.tile([128, 2, T], F32, tag="psA", name="ps_z")
            for k in range(2):
                ffn1_matmuls(ps[:, k], fp * 2 + k)
            nc.scalar.activation(
                zT[:, fp * 2 : fp * 2 + 2].rearrange("p a b -> p (a b)"),
                ps.rearrange("p a b -> p (a b)"),
                AF.Relu,
            )
            if FINE_YIELD and fp == 2:
                yield
    else:
        for ft in range(NF):
            ps = psA.tile([128, T], F32, tag="psA", name="ps_z1")
            ffn1_matmuls(ps[:], ft)
            nc.scalar.activation(
                zT[:, ft], ps[:], AF.Relu, bias=cst["b1e"][:, ft : ft + 1]
            )

    yield
    out_sb = xpool.tile([128, NT, C], F32, tag="out", name="out_sb")
    for tt in range(NT):
        ps = psB.tile([128, C], F32, tag="psB", name="ps_o")
        for fc in range(0, NF, 2):
            nc.tensor.matmul(
                ps[:],
                zT[:, fc : fc + 2, tt * 128 : (tt + 1) * 128],
                cst["w2"][:, fc : fc + 2],
                start=(fc == 0),
                stop=(fc == NF - 2),
                perf_mode=DR,
            )
        if int(os.environ.get("KERNEL_RESID_POOL", "0")):
            ocp = spool.tile([128, NT, C], F32, tag="ocp", name="ocp")
            nc.scalar.copy(ocp[:, tt], ps[:])
            nc.gpsimd.tensor_add(out_sb[:, tt], x2[:, tt], ocp[:, tt])
        else:
            nc.vector.tensor_add(out_sb[:, tt], x2[:, tt], ps[:])
        if flags["b2_nz"]:
            nc.vector.tensor_add(out_sb[:, tt], out_sb[:, tt], cst["b2_bc"][:])
    # store via the (idle) Pool engine's DGE queue: a store DMA waiting on
    # out_sb would head-of-line-block later x loads on the SP sequencer
    st_eng = {"sync": nc.sync, "gpsimd": nc.gpsimd, "vector": nc.vector,
              "scalar": nc.scalar}[os.environ.get("KERNEL_ST_ENG", "sync")]
    st_eng.dma_start(out_d[b].rearrange("(tt p) c -> p tt c", p=128), out_sb[:])
    yield


def _build_program(flags):
    nc = bacc.Bacc("TRN2", target_bir_lowering=False, debug=False)

    x_d = nc.dram_tensor("x_shard", (BPC, T, C), F32, kind="ExternalInput")
    out_d = nc.dram_tensor("out", (BPC, T, C), F32, kind="ExternalOutput")
    # weights are packed host-side into one blob per dtype: fewer external
    # tensors means fewer per-execution buffer bindings (and less per-step
    # dispatch overhead in any harness driving the NEFF)
    dram = {
        "wfp8": nc.dram_tensor("wfp8", (128, F8_TOT), F8, kind="ExternalInput"),
        "wbf16": nc.dram_tensor("wbf16", (128, BF_TOT), BF, kind="ExternalInput"),
    }
    need_f32 = flags["bq_nz"] or flags["bk_nz"] or flags["b1_nz"]
    if need_f32:
        dram["wf32"] = nc.dram_tensor("wf32", (128, F32_TOT), F32, kind="ExternalInput")
    b_specs = {}
    if flags["bv_nz"]:
        b_specs["bv_bc"] = (128, C)
    if flags["bp_nz"]:
        b_specs["bp_bc"] = (128, C)
    if flags["b2_nz"]:
        b_specs["b2_bc"] = (128, C)
    for name, shape in b_specs.items():
        dram[name] = nc.dram_tensor(name, shape, F32, kind="ExternalInput")

    with tile.TileContext(nc) as tc, ExitStack() as ctx:
        wpool = ctx.enter_context(tc.tile_pool(name="weights", bufs=1))
        xpool = ctx.enter_context(tc.tile_pool(name="xpool", bufs=int(os.environ.get("KERNEL_SBUFS", "5"))))
        apool = ctx.enter_context(tc.tile_pool(name="apool", bufs=int(os.environ.get("KERNEL_SBUFS", "5"))))
        spool = ctx.enter_context(tc.tile_pool(name="spool", bufs=int(os.environ.get("KERNEL_SBUFS", "5"))))
        pb = [int(v) for v in os.environ.get("KERNEL_PSUM", "2,2,2,2").split(",")]
        psA = ctx.enter_context(tc.tile_pool(name="psA", bufs=pb[0], space="PSUM"))
        psB = ctx.enter_context(tc.tile_pool(name="psB", bufs=pb[1], space="PSUM"))
        psS = ctx.enter_context(tc.tile_pool(name="psS", bufs=pb[2], space="PSUM"))
        psT = ctx.enter_context(tc.tile_pool(name="psT", bufs=pb[3], space="PSUM"))

        cst = {}
        blobs = {}
        for name in dram:
            t = wpool.tile(list(dram[name].shape), dram[name].dtype, tag=name, name=name + "_sb")
            blobs[name] = t
            if name not in ("wfp8",):
                nc.sync.dma_start(t[:], dram[name][:])
            else:
                # wq/wk/wv (the tail of the fp8 blob) are not needed until
                # after the first LN+transpose; defer them one round so they
                # don't delay the first x loads
                nc.sync.dma_start(t[:, :F8_MID], dram[name][:, :F8_MID])
            if name in b_specs:
                cst[name] = t

        def _load_mid():
            t = blobs["wfp8"]
            nc.sync.dma_start(t[:, F8_MID:], dram["wfp8"][:, F8_MID:])

        def seg(tile, off, dims):
            base = tile[:]
            strides = []
            s = 1
            for d in reversed(dims):
                strides.append(s)
                s *= d
            strides = list(reversed(strides))
            return bass.AP(
                tensor=base.tensor,
                offset=base.offset + off,
                ap=[base.ap[0]] + [[st, d] for st, d in zip(strides, dims)],
            )

        for name, (off, dims) in F8_SEGS.items():
            cst[name] = seg(blobs["wfp8"], off, dims)
        for name, (off, dims) in BF_SEGS.items():
            cst[name] = seg(blobs["wbf16"], off, dims)
        for name, (off, dims) in F32_SEGS.items():
            cst[name] = seg(blobs["wf32"], off, dims) if need_f32 else None
        eps = wpool.tile([128, 1], F32, tag="eps", name="eps_sb")
        nc.vector.memset(eps[:], LN_EPS)
        cst["eps"] = eps

        pools = (wpool, xpool, apool, spool, psA, psB, psS, psT)
        depth = int(os.environ.get("KERNEL_ILV", "8"))
        stagger = int(os.environ.get("KERNEL_STAGGER", "1"))
        gens = [
            _emit_seq(nc, tc, pools, cst, b, x_d, out_d, flags)
            for b in range(BPC)
        ]
        live = []
        nxt = 0
        rnd = 0
        mid_loaded = False
        while live or nxt < BPC:
            if rnd == 1 and not mid_loaded:
                _load_mid()
                mid_loaded = True
            if nxt < BPC and len(live) < depth and rnd % max(stagger, 1) == 0:
                live.append(nxt)
                nxt += 1
            for b in list(live):
                if next(gens[b], "end") == "end":
                    live.remove(b)
            rnd += 1

    nc.compile()
    _dedupe_act_table_loads(nc)
    _dedupe_ldweights(nc)
    return nc


def _dedupe_ldweights(nc):
    """Drop InstLdweights that reload the stationary already in the PE array.

    The PE keeps its stationary operand across matmuls, so a Ldweights that is
    identical (same tensor/offset/AP/dtype/perf-mode/transpose/tile placement)
    to the previous one in the PE stream with only Matmults in between is a
    no-op. Restricted to write-once constant tensors (identity / ones / mask)
    so an SBUF ring-buffer reuse can never invalidate the loaded weights; the
    dropped instruction must carry no semaphore waits/updates.
    """
    if not int(os.environ.get("KERNEL_LDW_DEDUP", "1")):
        return
    const_sets = ("wbf16_sb",)

    def sig(inst):
        a = inst.ins[0]
        return (
            a.memsetref,
            a.memref,
            a.offset,
            tuple(map(tuple, a.ap)),
            str(a.dtype),
            str(inst.perf_mode),
            inst.is_transpose,
            inst.tile_position,
            inst.tile_size,
        )

    dropped = 0
    for blk in nc.m.functions[0].blocks:
        last = None
        keep = []
        for inst in blk.instructions:
            eng = getattr(inst, "engine", None)
            if eng != mybir.EngineType.PE:
                keep.append(inst)
                continue
            if isinstance(inst, mybir.InstLdweights):
                s = sig(inst)
                if (
                    last == s
                    and inst.sync_info is None
                    and any(str(s[0]).startswith(c) for c in const_sets)
                ):
                    dropped += 1
                    continue
                last = s
                keep.append(inst)
            elif isinstance(inst, mybir.InstMatmult):
                keep.append(inst)
            else:
                last = None
                keep.append(inst)
        if len(keep) != len(blk.instructions):
            blk.instructions[:] = keep


def _dedupe_act_table_loads(nc):
    """All ACT funcs in this kernel live in one act table; rewrite the
    auto-inserted per-function table loads to a single load of that table."""
    if not int(os.environ.get("KERNEL_ACT_DEDUP", "1")):
        return
    try:
        from concourse.hw_specs import get_activation_tables

        tabs = get_activation_tables(nc.m.arch)
        need = {AF.Exp, AF.Ln, AF.Relu, AF.Copy, AF.Identity}
        combined = next(
            i for i, fs in enumerate(tabs.values()) if need <= fs
        )
    except Exception:
        return
    for blk in nc.m.functions[0].blocks:
        first = True
        keep = []
        for inst in blk.instructions:
            if isinstance(inst, mybir.InstLoadActFuncSet) and inst.sync_info is None:
                if first:
                    inst.act_func_set_id = combined
                    first = False
                    keep.append(inst)
                continue
            keep.append(inst)
        if len(keep) != len(blk.instructions):
            blk.instructions[:] = keep


def _prepare_host_inputs(x, Wq, Wk, Wv, Wp, bp, W1, b1, W2, b2, g1, be1, g2, be2):
    f = np.float32
    x = np.asarray(x, f)
    Wq = np.asarray(Wq, f)
    Wk = np.asarray(Wk, f)
    Wv = np.asarray(Wv, f)
    Wp = np.asarray(Wp, f)
    W1 = np.asarray(W1, f)
    W2 = np.asarray(W2, f)
    bp = np.asarray(bp, f)
    b1 = np.asarray(b1, f)
    b2 = np.asarray(b2, f)
    g1 = np.asarray(g1, f)
    be1 = np.asarray(be1, f)
    g2 = np.asarray(g2, f)
    be2 = np.asarray(be2, f)

    # stack per-head QKV weights: (H, C, DH) -> (C, C) with d = h*DH + dh
    Wq_all = np.transpose(Wq, (1, 0, 2)).reshape(C, C)
    Wk_all = np.transpose(Wk, (1, 0, 2)).reshape(C, C)
    Wv_all = np.transpose(Wv, (1, 0, 2)).reshape(C, C)

    scale = 1.0 / np.sqrt(np.float32(C))
    Wq_eff = (g1[:, None] * Wq_all) * scale
    bq = (be1 @ Wq_all) * scale
    Wk_eff = g1[:, None] * Wk_all
    bk = be1 @ Wk_all
    Wv_eff = g1[:, None] * Wv_all
    bv = be1 @ Wv_all
    W1_eff = g2[:, None] * W1
    b1e = b1 + be2 @ W1

    def chunk_k(w, nk, dt=FP8):  # (K, N) -> (128, nk, N)
        K, N = w.shape
        return np.ascontiguousarray(
            w.reshape(nk, 128, N).transpose(1, 0, 2).astype(dt)
        )

    flags = {
        "bq_nz": bool(np.any(bq != 0)),
        "bk_nz": bool(np.any(bk != 0)),
        "bv_nz": bool(np.any(bv != 0)),
        "bp_nz": bool(np.any(bp != 0)),
        "b1_nz": bool(np.any(b1e != 0)),
        "b2_nz": bool(np.any(b2 != 0)),
    }
    # transposed-score mask: (s0,t0) triu | (s0,t1) ones | (s1,t1) triu
    tri = np.triu(np.ones((128, 128), np.float32))
    cmaskT = np.concatenate([tri, np.ones((128, 128), np.float32), tri], axis=1)

    parts8 = {
        "wq": chunk_k(Wq_eff, NC_),
        "wk": chunk_k(Wk_eff, NC_),
        "wv": chunk_k(Wv_eff, NC_),
        "wp": chunk_k(Wp, NC_),
        "w1": chunk_k(W1_eff, NC_),
        "w2": chunk_k(W2, NF),
    }
    partsbf = {
        "ident": np.eye(128, dtype=np.float32).astype(BF16),
        "cmaskT": np.ascontiguousarray(cmaskT.astype(BF16)),
        "ones64": np.ones((128, 64), dtype=np.float32).astype(BF16),
        # -30 on causally-masked (s > t) positions, added to scores pre-exp
        "trineg": np.ascontiguousarray(
            (-30.0 * (1.0 - tri)).astype(BF16)
        ),
    }
    partsf32 = {
        "bq": np.ascontiguousarray(bq.reshape(NC_, 128).T.astype(f)),
        "bk": np.ascontiguousarray(bk.reshape(NC_, 128).T.astype(f)),
        "b1e": np.ascontiguousarray(b1e.reshape(NF, 128).T.astype(f)),
    }

    def pack(parts, segs, total, dt):
        blob = np.zeros((128, total), dtype=dt)
        for name, (off, dims) in segs.items():
            n = int(np.prod(dims))
            blob[:, off : off + n] = parts[name].reshape(128, n)
        return blob

    common = {
        "wfp8": pack(parts8, F8_SEGS, F8_TOT, FP8),
        "wbf16": pack(partsbf, BF_SEGS, BF_TOT, BF16),
    }
    if flags["bq_nz"] or flags["bk_nz"] or flags["b1_nz"]:
        common["wf32"] = pack(partsf32, F32_SEGS, F32_TOT, f)
    if flags["bv_nz"]:
        common["bv_bc"] = np.ascontiguousarray(np.broadcast_to(bv, (128, C)).astype(f))
    if flags["bp_nz"]:
        common["bp_bc"] = np.ascontiguousarray(np.broadcast_to(bp, (128, C)).astype(f))
    if flags["b2_nz"]:
        common["b2_bc"] = np.ascontiguousarray(np.broadcast_to(b2, (128, C)).astype(f))
    return x, common, flags


def kernel(x, Wq, Wk, Wv, Wp, bp, W1, b1, W2, b2, g1, be1, g2, be2):
    global LAST_EXEC_NS, LAST_RESULTS
    x, common, flags = _prepare_host_inputs(
        x, Wq, Wk, Wv, Wp, bp, W1, b1, W2, b2, g1, be1, g2, be2
    )
    key = tuple(sorted(flags.items()))
    if key not in _PROGRAM_CACHE:
        _PROGRAM_CACHE[key] = _build_program(flags)
    nc = _PROGRAM_CACHE[key]

    in_maps = []
    for c in range(N_CORES):
        m = dict(common)
        m["x_shard"] = np.ascontiguousarray(x[c * BPC : (c + 1) * BPC])
        in_maps.append(m)

    kwargs = {}
    if PROFILE:
        kwargs["trace"] = True
        if TRACE_DIR:
            kwargs["tmpdir"] = TRACE_DIR
    res = run_bass_kernel_spmd(nc, in_maps, core_ids=list(range(N_CORES)), **kwargs)
    LAST_EXEC_NS = res.exec_time_ns
    LAST_RESULTS = res
    out = np.concatenate([np.asarray(r["out"]) for r in res.results], axis=0)
    return out.astype(np.float32)



# revision 29
# speedup vs baseline: 1.1607x; 1.1607x over previous
"""Trainium2 Bass kernel for a dense transformer block (pre-LN, 6-head causal
attention, 4x FFN) over x:(128,256,384) f32.

Strategy: pure data-parallel over batch across 8 NeuronCores (16 sequences per
core). Per-core Tile kernel computes the whole block per sequence:
  LN1 -> QKV (bf16 matmuls, LN scale/shift folded into weights on host)
  -> causal softmax (no max-subtract; scores are tiny by construction)
  -> P@V via PE transposes -> proj + residual -> LN2 -> FFN (relu) + residual.
All matmul operands are bf16 (fp32 PSUM accumulation); the residual stream
stays fp32 end-to-end. ACT stays on one function table (ln/exp/copy/relu);
rsqrt is computed as exp(-0.5*ln(var+eps)).
"""

import os
import sys

import numpy as np

try:
    import concourse.bass as bass  # noqa: F401
except ImportError:
    sys.path.insert(0, "/opt/trn_rl_repo")

import ml_dtypes
from contextlib import ExitStack

import concourse.bass as bass
import concourse.tile as tile
from concourse import bacc, mybir
from concourse.bass_utils import run_bass_kernel_spmd

BF16 = ml_dtypes.bfloat16
FP8 = ml_dtypes.float8_e4m3

N_CORES = 8
B, T, C = 128, 256, 384
H, DH = 6, 64
F = 4 * C  # 1536
BPC = B // N_CORES  # sequences per core
NT = T // 128  # 2 t-tiles
NC_ = C // 128  # 3 c-chunks
NF = F // 128  # 12 f-chunks
LN_EPS = 1e-5

AF = mybir.ActivationFunctionType
ALU = mybir.AluOpType
F32 = mybir.dt.float32
BF = mybir.dt.bfloat16
F8 = mybir.dt.float8e4
DR = mybir.MatmulPerfMode.DoubleRow

# packed-weight blob layouts: name -> (element offset, free-dim shape)
F8_SEGS = {
    "wp": (0, (NC_, C)),
    "w1": (NC_ * C, (NC_, F)),
    "w2": (NC_ * C + NC_ * F, (NF, C)),
    "wq": (NC_ * C + NC_ * F + NF * C, (NC_, C)),
    "wk": (2 * NC_ * C + NC_ * F + NF * C, (NC_, C)),
    "wv": (3 * NC_ * C + NC_ * F + NF * C, (NC_, C)),
}
F8_MID = NC_ * C + NC_ * F + NF * C  # wq/wk/wv tail deferred one round
F8_TOT = 4 * NC_ * C + NC_ * F + NF * C
BF_SEGS = {
    "ident": (0, (128,)),
    "ones64": (128, (64,)),
    "trineg": (192, (128,)),
    "cmaskT": (320, (C,)),
}
BF_TOT = 320 + C
F32_SEGS = {
    "bq": (0, (NC_,)),
    "bk": (NC_, (NC_,)),
    "b1e": (2 * NC_, (NF,)),
}
F32_TOT = 2 * NC_ + NF

SB_SHORT = int(os.environ.get("KERNEL_SBUFS_SHORT", "0")) or None
FINE_YIELD = int(os.environ.get("KERNEL_FINE_YIELD", "0"))
COARSE_YIELD = int(os.environ.get("KERNEL_COARSE_YIELD", "0"))

_PROGRAM_CACHE = {}
LAST_EXEC_NS = None
LAST_RESULTS = None
PROFILE = bool(int(os.environ.get("KERNEL_PROFILE", "0")))
DMA_T_XN = bool(int(os.environ.get("KERNEL_DMA_T_XN", "0")))
ATPACK = bool(int(os.environ.get("KERNEL_ATPACK", "1")))
DMA_T_P = bool(int(os.environ.get("KERNEL_DMA_T_P", "0")))
TRACE_DIR = os.environ.get("KERNEL_TRACE_DIR") or None


def _bcast_h(ap, n):
    """Insert a stride-0 dim of size n after the partition dim of a 2D AP."""
    return bass.AP(
        tensor=ap.tensor, offset=ap.offset, ap=[ap.ap[0], [0, n], ap.ap[1]]
    )


def _emit_seq(nc, tc, pools, cst, b, x_d, out_d, flags):
    """Emit IR for one sequence b."""
    (wpool, xpool, apool, spool, psA, psB, psS, psT) = pools

    # ---- load x (T-major: partition = t%128) ----
    x_sb = xpool.tile([128, NT, C], F32, tag="x", name="x_sb")
    nc.sync.dma_start(x_sb[:], x_d[b].rearrange("(tt p) c -> p tt c", p=128))

    def layer_norm_to_bf16(src_sb, tag):
        """bn_stats/aggr per t-tile; returns (x - mu) * rstd as bf16.
        rstd = exp(-0.5 * ln(var + eps)) keeps ACT on the ln/exp table."""
        st = spool.tile([128, NT, 6], F32, tag=f"st{tag}", name="st")
        mv = spool.tile([128, NT, 2], F32, tag=f"mv{tag}", name="mv")
        for tt in range(NT):
            nc.vector.bn_stats(st[:, tt], src_sb[:, tt])
            nc.vector.bn_aggr(mv[:, tt], st[:, tt])
        rstd = spool.tile([128, NT], F32, tag=f"rstd{tag}", name="rstd")
        if int(os.environ.get("KERNEL_RSTD_POOL", "0")):
            # Newton rsqrt on the (idle) Pool engine; var+eps is within
            # [0.5, 2] for LN of ~unit-variance rows, so y0 = 1.5 - 0.5v
            # converges to <0.1% in 3 iterations.
            v = mv[:, :, 1]
            tmp = spool.tile([128, NT], F32, tag=f"nt{tag}", name="nt")
            nc.gpsimd.tensor_scalar(
                out=rstd[:], in0=v, scalar1=-0.5, scalar2=1.5,
                op0=ALU.mult, op1=ALU.add,
            )
            for _ in range(3):
                nc.gpsimd.tensor_mul(tmp[:], rstd[:], rstd[:])
                nc.gpsimd.tensor_mul(tmp[:], tmp[:], v)
                nc.gpsimd.tensor_scalar(
                    out=tmp[:], in0=tmp[:], scalar1=-0.5, scalar2=1.5,
                    op0=ALU.mult, op1=ALU.add,
                )
                nc.gpsimd.tensor_mul(rstd[:], rstd[:], tmp[:])
        else:
            lnv = spool.tile([128, NT], F32, tag=f"lnv{tag}", name="lnv")
            nc.scalar.activation(lnv[:], mv[:, :, 1], AF.Ln, bias=cst["eps"][:, 0:1])
            nc.scalar.activation(rstd[:], lnv[:], AF.Exp, scale=-0.5)
        xn = xpool.tile([128, NT, C], BF, tag=f"xn{tag}", name="xn")
        for tt in range(NT):
            nc.gpsimd.tensor_scalar(
                out=xn[:, tt],
                in0=src_sb[:, tt],
                scalar1=mv[:, tt, 0:1],
                scalar2=rstd[:, tt : tt + 1],
                op0=ALU.subtract,
                op1=ALU.mult,
            )
        return xn

    def transpose_to_cmajor(xn, tag):
        """[128, NT, C] bf16 -> [128, NC_, T] fp8 via PE transposes (bf16)
        with the fp8 cast folded into the PSUM->SBUF copy."""
        xnT = xpool.tile([128, NC_, T], F8, tag=f"xnT{tag}", name="xnT")
        if DMA_T_XN:
            for cc in range(NC_):
                for tt in range(NT):
                    nc.sync.dma_start_transpose(
                        xnT[:, cc, tt * 128 : (tt + 1) * 128],
                        xn[:, tt, cc * 128 : (cc + 1) * 128],
                    )
            return xnT
        tp = psT.tile([128, NC_, T], BF, tag="tps", name="tp")
        for cc in range(NC_):
            for tt in range(NT):
                nc.tensor.transpose(
                    tp[:, cc, tt * 128 : (tt + 1) * 128],
                    xn[:, tt, cc * 128 : (cc + 1) * 128],
                    cst["ident"][:],
                )
        nc.vector.tensor_copy(xnT[:], tp[:])
        return xnT

    xn1 = layer_norm_to_bf16(x_sb, "1")
    xnT = transpose_to_cmajor(xn1, "1")
    yield

    # ---- QKV projections (C-major Q/K, T-major V) ----
    # Q and K share one tile so their dt=2 PSUM copies merge into a single
    # (strided) ACT instruction
    QKT = apool.tile([128, 2, NC_, T], BF, tag="QT", name="QKT", bufs=SB_SHORT)
    QT, KT = QKT[:, 0], QKT[:, 1]
    ps2 = psB.tile([128, 2, T], F32, tag="psB", name="ps_qk2")
    for qi, (w_sb, b_sb, b_nz) in enumerate((
        (cst["wq"], cst["bq"], flags["bq_nz"]),
        (cst["wk"], cst["bk"], flags["bk_nz"]),
    )):
        dst = QKT[:, qi]
        ps01 = psA.tile([128, 2, T], F32, tag="psA", name="ps_qk01")
        for dt in range(NC_):
            ps = ps01[:, dt] if dt < 2 else ps2[:, qi]
            nc.tensor.matmul(
                ps,
                w_sb[:, 0:2, dt * 128 : (dt + 1) * 128],
                xnT[:, 0:2],
                start=True,
                stop=False,
                perf_mode=DR,
            )
            nc.tensor.matmul(
                ps,
                w_sb[:, 2, dt * 128 : (dt + 1) * 128],
                xnT[:, 2],
                start=False,
                stop=True,
            )
        if b_nz:
            for dt in range(NC_):
                ps = ps01[:, dt] if dt < 2 else ps2[:, qi]
                nc.scalar.activation(
                    dst[:, dt], ps, AF.Identity, bias=b_sb[:, dt : dt + 1]
                )
        else:
            nc.scalar.copy(dst[:, 0:2], ps01[:])
    if not flags["bq_nz"] and not flags["bk_nz"]:
        nc.scalar.copy(QKT[:, :, 2], ps2[:])
    elif not flags["bq_nz"]:
        nc.scalar.copy(QKT[:, 0, 2], ps2[:, 0])
    elif not flags["bk_nz"]:
        nc.scalar.copy(QKT[:, 1, 2], ps2[:, 1])
    V = apool.tile([128, NT, C], BF, tag="V", name="V", bufs=SB_SHORT)
    for tt in range(NT):
        ps = psB.tile([128, C], F32, tag="psB", name="ps_v")
        nc.tensor.matmul(
            ps[:],
            xnT[:, 0:2, tt * 128 : (tt + 1) * 128],
            cst["wv"][:, 0:2],
            start=True,
            stop=False,
            perf_mode=DR,
        )
        nc.tensor.matmul(
            ps[:],
            xnT[:, 2, tt * 128 : (tt + 1) * 128],
            cst["wv"][:, 2],
            start=False,
            stop=True,
        )
        if flags["bv_nz"]:
            nc.vector.tensor_add(ps[:], ps[:], cst["bv_bc"][:])
        nc.vector.tensor_copy(V[:, tt], ps[:])
    if not COARSE_YIELD:
        yield

    # ---- attention (transposed-score form) ----
    # ET layout per head: [128, H, 384]: cols 0:256 = (s0 x t0..t1), 256:384
    # = (s1 x t1). Scores come out pre-transposed (K stationary, Q moving),
    # so P never needs a PE transpose; softmax denominators are computed by
    # ones-matmuls replicated across 64 partitions, and the normalization is
    # applied AFTER P@V (it commutes with the V contraction).
    ET = apool.tile([128, H, 384], BF, tag="E", name="ET", bufs=SB_SHORT)
    rec = spool.tile([128, NC_, T], F32, tag="rec", name="rec", bufs=SB_SHORT)
    attnT = apool.tile([128, NC_, T], F8, tag="attnT", name="attnT", bufs=SB_SHORT)
    mask_mode = os.environ.get("KERNEL_MASK", "pe")

    for h in range(H):
        dt, off = h // 2, (h % 2) * 64
        sc = psS.tile([128, 384], F32, tag="sc", name="sc")
        nc.tensor.matmul(
            sc[:, 0:256],
            KT[off : off + 64, dt, 0:128],
            QT[off : off + 64, dt, :],
            start=True,
            stop=True,
        )
        nc.tensor.matmul(
            sc[:, 256:384],
            KT[off : off + 64, dt, 128:256],
            QT[off : off + 64, dt, 128:256],
            start=True,
            stop=True,
        )
        if mask_mode == "pe":
            # add -30 to causally-masked positions so exp() flushes them to
            # ~1e-13 (constant-matrix add via identity matmul; no DVE pass)
            nc.tensor.matmul(
                sc[:, 0:128], cst["ident"][:], cst["trineg"][:],
                start=False, stop=True, skip_group_check=True,
            )
            nc.tensor.matmul(
                sc[:, 256:384], cst["ident"][:], cst["trineg"][:],
                start=False, stop=True, skip_group_check=True,
            )
        nc.scalar.activation(ET[:, h], sc[:], AF.Exp)
        if mask_mode in ("pool", "dve") and h % 2 == 1:
            # zero the causally-masked positions by multiplying the two
            # triangular blocks of this head pair (cols 0:128 and 256:384)
            # with the triu mask, broadcast from cmaskT's first 128 cols
            et0 = ET[:, h - 1]
            blocks = bass.AP(
                tensor=et0.tensor,
                offset=et0.offset,
                ap=[et0.ap[0], [384, 2], [256, 2], [1, 128]],
            )
            cm = cst["cmaskT"][:]
            tri_bc = bass.AP(
                tensor=cm.tensor,
                offset=cm.offset,
                ap=[cm.ap[0], [0, 2], [0, 2], [1, 128]],
            )
            eng = nc.gpsimd if mask_mode == "pool" else nc.vector
            eng.tensor_mul(blocks, blocks, tri_bc)
        if FINE_YIELD and h == 2:
            yield
    if COARSE_YIELD < 2:
        yield

    for pair in range(H // 2):
        # softmax denominators for heads (2*pair, 2*pair+1), replicated
        # across 64 partitions each via ones-matmul
        sm = psT.tile([128, T], F32, tag="tps", name="sums")
        for s_i in range(2):
            h = pair * 2 + s_i
            tp_kw = {} if s_i == 0 else {"tile_position": (0, 64)}
            sl = slice(s_i * 64, (s_i + 1) * 64)
            nc.tensor.matmul(
                sm[sl, 0:256], cst["ones64"][:], ET[:, h, 0:256],
                start=True, stop=False, skip_group_check=True, **tp_kw,
            )
            nc.tensor.matmul(
                sm[sl, 128:256], cst["ones64"][:], ET[:, h, 256:384],
                start=False, stop=True, skip_group_check=True, **tp_kw,
            )
        nc.vector.reciprocal(rec[:, pair], sm[:])

        at = psT.tile([128, 256], F32, tag="tps", name="atp")
        for s_i in range(2):
            h = pair * 2 + s_i
            dh = h * DH
            tp_kw = {} if s_i == 0 else {"tile_position": (0, 64)}
            sl = slice(s_i * 64, (s_i + 1) * 64)
            nc.tensor.matmul(
                at[sl, 0:256], V[:, 0, dh : dh + 64], ET[:, h, 0:256],
                start=True, stop=False, skip_group_check=True, **tp_kw,
            )
            nc.tensor.matmul(
                at[sl, 128:256], V[:, 1, dh : dh + 64], ET[:, h, 256:384],
                start=False, stop=True, skip_group_check=True, **tp_kw,
            )
        # normalize (divide by per-(head,t) denominator) + cast to fp8
        nc.vector.tensor_mul(attnT[:, pair], at[:], rec[:, pair])
        if FINE_YIELD and pair == 0:
            yield
    yield

    # ---- projection + residual ----
    x2 = xpool.tile([128, NT, C], F32, tag="x2", name="x2")
    for tt in range(NT):
        ps = psB.tile([128, C], F32, tag="psB", name="ps_proj")
        nc.tensor.matmul(
            ps[:],
            attnT[:, 0:2, tt * 128 : (tt + 1) * 128],
            cst["wp"][:, 0:2],
            start=True,
            stop=False,
            perf_mode=DR,
        )
        nc.tensor.matmul(
            ps[:],
            attnT[:, 2, tt * 128 : (tt + 1) * 128],
            cst["wp"][:, 2],
            start=False,
            stop=True,
        )
        if int(os.environ.get("KERNEL_RESID_POOL", "0")):
            ycp = spool.tile([128, NT, C], F32, tag="ycp", name="ycp")
            nc.scalar.copy(ycp[:, tt], ps[:])
            nc.gpsimd.tensor_add(x2[:, tt], x_sb[:, tt], ycp[:, tt])
        else:
            nc.vector.tensor_add(x2[:, tt], x_sb[:, tt], ps[:])
        if flags["bp_nz"]:
            nc.vector.tensor_add(x2[:, tt], x2[:, tt], cst["bp_bc"][:])

    # ---- LN2 + FFN ----
    xn2 = layer_norm_to_bf16(x2, "2")
    xn2T = transpose_to_cmajor(xn2, "2")
    yield

    zT = apool.tile([128, NF, T], F8, tag="zT", name="zT", bufs=SB_SHORT)

    def ffn1_matmuls(ps, ft):
        nc.tensor.matmul(
            ps,
            cst["w1"][:, 0:2, ft * 128 : (ft + 1) * 128],
            xn2T[:, 0:2],
            start=True,
            stop=False,
            perf_mode=DR,
        )
        nc.tensor.matmul(
            ps,
            cst["w1"][:, 2, ft * 128 : (ft + 1) * 128],
            xn2T[:, 2],
            start=False,
            stop=True,
        )

    if not flags["b1_nz"]:
        # paired f-tiles: one [128,512] psum bank, one relu per pair
        for fp in range(NF // 2):
            ps = psA.tile([128, 2, T], F32, tag="psA", name="ps_z")
            for k in range(2):
                ffn1_matmuls(ps[:, k], fp * 2 + k)
            nc.scalar.activation(
                zT[:, fp * 2 : fp * 2 + 2].rearrange("p a b -> p (a b)"),
                ps.rearrange("p a b -> p (a b)"),
                AF.Relu,
            )
            if FINE_YIELD and fp == 2:
                yield
    else:
        for ft in range(NF):
            ps = psA.tile([128, T], F32, tag="psA", name="ps_z1")
            ffn1_matmuls(ps[:], ft)
            nc.scalar.activation(
                zT[:, ft], ps[:], AF.Relu, bias=cst["b1e"][:, ft : ft + 1]
            )

    yield
    out_sb = xpool.tile([128, NT, C], F32, tag="out", name="out_sb")
    for tt in range(NT):
        ps = psB.tile([128, C], F32, tag="psB", name="ps_o")
        for fc in range(0, NF, 2):
            nc.tensor.matmul(
                ps[:],
                zT[:, fc : fc + 2, tt * 128 : (tt + 1) * 128],
                cst["w2"][:, fc : fc + 2],
                start=(fc == 0),
                stop=(fc == NF - 2),
                perf_mode=DR,
            )
        if int(os.environ.get("KERNEL_RESID_POOL", "0")):
            ocp = spool.tile([128, NT, C], F32, tag="ocp", name="ocp")
            nc.scalar.copy(ocp[:, tt], ps[:])
            nc.gpsimd.tensor_add(out_sb[:, tt], x2[:, tt], ocp[:, tt])
        else:
            nc.vector.tensor_add(out_sb[:, tt], x2[:, tt], ps[:])
        if flags["b2_nz"]:
            nc.vector.tensor_add(out_sb[:, tt], out_sb[:, tt], cst["b2_bc"][:])
    # store via the (idle) Pool engine's DGE queue: a store DMA waiting on
    # out_sb would head-of-line-block later x loads on the SP sequencer
    st_eng = {"sync": nc.sync, "gpsimd": nc.gpsimd, "vector": nc.vector,
              "scalar": nc.scalar}[os.environ.get("KERNEL_ST_ENG", "sync")]
    st_eng.dma_start(out_d[b].rearrange("(tt p) c -> p tt c", p=128), out_sb[:])
    yield


def _build_program(flags):
    nc = bacc.Bacc("TRN2", target_bir_lowering=False, debug=False)

    x_d = nc.dram_tensor("x_shard", (BPC, T, C), F32, kind="ExternalInput")
    out_d = nc.dram_tensor("out", (BPC, T, C), F32, kind="ExternalOutput")
    # weights are packed host-side into one blob per dtype: fewer external
    # tensors means fewer per-execution buffer bindings (and less per-step
    # dispatch overhead in any harness driving the NEFF)
    # single packed weight input: fp8 blob bytes followed by the bf16
    # constants' bytes (bitcast on load); fewer external tensors means fewer
    # per-execution buffer bindings
    dram = {
        "wfp8": nc.dram_tensor(
            "wfp8", (128, F8_TOT + 2 * BF_TOT), F8, kind="ExternalInput"
        ),
    }
    need_f32 = flags["bq_nz"] or flags["bk_nz"] or flags["b1_nz"]
    if need_f32:
        dram["wf32"] = nc.dram_tensor("wf32", (128, F32_TOT), F32, kind="ExternalInput")
    b_specs = {}
    if flags["bv_nz"]:
        b_specs["bv_bc"] = (128, C)
    if flags["bp_nz"]:
        b_specs["bp_bc"] = (128, C)
    if flags["b2_nz"]:
        b_specs["b2_bc"] = (128, C)
    for name, shape in b_specs.items():
        dram[name] = nc.dram_tensor(name, shape, F32, kind="ExternalInput")

    with tile.TileContext(nc) as tc, ExitStack() as ctx:
        wpool = ctx.enter_context(tc.tile_pool(name="weights", bufs=1))
        xpool = ctx.enter_context(tc.tile_pool(name="xpool", bufs=int(os.environ.get("KERNEL_SBUFS", "5"))))
        apool = ctx.enter_context(tc.tile_pool(name="apool", bufs=int(os.environ.get("KERNEL_SBUFS", "5"))))
        spool = ctx.enter_context(tc.tile_pool(name="spool", bufs=int(os.environ.get("KERNEL_SBUFS", "5"))))
        pb = [int(v) for v in os.environ.get("KERNEL_PSUM", "2,2,2,2").split(",")]
        psA = ctx.enter_context(tc.tile_pool(name="psA", bufs=pb[0], space="PSUM"))
        psB = ctx.enter_context(tc.tile_pool(name="psB", bufs=pb[1], space="PSUM"))
        psS = ctx.enter_context(tc.tile_pool(name="psS", bufs=pb[2], space="PSUM"))
        psT = ctx.enter_context(tc.tile_pool(name="psT", bufs=pb[3], space="PSUM"))

        cst = {}
        blobs = {}
        blobs["wfp8"] = wpool.tile([128, F8_TOT], F8, tag="wfp8", name="wfp8_sb")
        blobs["wbf16"] = wpool.tile([128, BF_TOT], BF, tag="wbf16", name="wbf16_sb")
        # bf16 constants ride in the tail bytes of the fp8 dram blob
        nc.sync.dma_start(
            blobs["wbf16"][:].bitcast(F8), dram["wfp8"][:, F8_TOT : F8_TOT + 2 * BF_TOT]
        )
        # wq/wk/wv (the tail of the fp8 segment) are not needed until after
        # the first LN+transpose; defer them one round so they don't delay
        # the first x loads
        nc.sync.dma_start(blobs["wfp8"][:, :F8_MID], dram["wfp8"][:, :F8_MID])
        if need_f32:
            t = wpool.tile([128, F32_TOT], F32, tag="wf32", name="wf32_sb")
            blobs["wf32"] = t
            nc.sync.dma_start(t[:], dram["wf32"][:])
        for name in b_specs:
            t = wpool.tile(list(dram[name].shape), F32, tag=name, name=name + "_sb")
            nc.sync.dma_start(t[:], dram[name][:])
            cst[name] = t

        def _load_mid():
            t = blobs["wfp8"]
            nc.sync.dma_start(t[:, F8_MID:], dram["wfp8"][:, F8_MID:F8_TOT])

        def seg(tile, off, dims):
            base = tile[:]
            strides = []
            s = 1
            for d in reversed(dims):
                strides.append(s)
                s *= d
            strides = list(reversed(strides))
            return bass.AP(
                tensor=base.tensor,
                offset=base.offset + off,
                ap=[base.ap[0]] + [[st, d] for st, d in zip(strides, dims)],
            )

        for name, (off, dims) in F8_SEGS.items():
            cst[name] = seg(blobs["wfp8"], off, dims)
        for name, (off, dims) in BF_SEGS.items():
            cst[name] = seg(blobs["wbf16"], off, dims)
        for name, (off, dims) in F32_SEGS.items():
            cst[name] = seg(blobs["wf32"], off, dims) if need_f32 else None
        eps = wpool.tile([128, 1], F32, tag="eps", name="eps_sb")
        nc.vector.memset(eps[:], LN_EPS)
        cst["eps"] = eps

        pools = (wpool, xpool, apool, spool, psA, psB, psS, psT)
        depth = int(os.environ.get("KERNEL_ILV", "8"))
        stagger = int(os.environ.get("KERNEL_STAGGER", "1"))
        gens = [
            _emit_seq(nc, tc, pools, cst, b, x_d, out_d, flags)
            for b in range(BPC)
        ]
        live = []
        nxt = 0
        rnd = 0
        mid_loaded = False
        while live or nxt < BPC:
            if rnd == 1 and not mid_loaded:
                _load_mid()
                mid_loaded = True
            if nxt < BPC and len(live) < depth and rnd % max(stagger, 1) == 0:
                live.append(nxt)
                nxt += 1
            for b in list(live):
                if next(gens[b], "end") == "end":
                    live.remove(b)
            rnd += 1

    nc.compile()
    _dedupe_act_table_loads(nc)
    _dedupe_ldweights(nc)
    return nc


def _dedupe_ldweights(nc):
    """Drop InstLdweights that reload the stationary already in the PE array.

    The PE keeps its stationary operand across matmuls, so a Ldweights that is
    identical (same tensor/offset/AP/dtype/perf-mode/transpose/tile placement)
    to the previous one in the PE stream with only Matmults in between is a
    no-op. Restricted to write-once constant tensors (identity / ones / mask)
    so an SBUF ring-buffer reuse can never invalidate the loaded weights; the
    dropped instruction must carry no semaphore waits/updates.
    """
    if not int(os.environ.get("KERNEL_LDW_DEDUP", "1")):
        return
    const_sets = ("wbf16_sb",)

    def sig(inst):
        a = inst.ins[0]
        return (
            a.memsetref,
            a.memref,
            a.offset,
            tuple(map(tuple, a.ap)),
            str(a.dtype),
            str(inst.perf_mode),
            inst.is_transpose,
            inst.tile_position,
            inst.tile_size,
        )

    dropped = 0
    for blk in nc.m.functions[0].blocks:
        last = None
        keep = []
        for inst in blk.instructions:
            eng = getattr(inst, "engine", None)
            if eng != mybir.EngineType.PE:
                keep.append(inst)
                continue
            if isinstance(inst, mybir.InstLdweights):
                s = sig(inst)
                if (
                    last == s
                    and inst.sync_info is None
                    and any(str(s[0]).startswith(c) for c in const_sets)
                ):
                    dropped += 1
                    continue
                last = s
                keep.append(inst)
            elif isinstance(inst, mybir.InstMatmult):
                keep.append(inst)
            else:
                last = None
                keep.append(inst)
        if len(keep) != len(blk.instructions):
            blk.instructions[:] = keep


def _dedupe_act_table_loads(nc):
    """All ACT funcs in this kernel live in one act table; rewrite the
    auto-inserted per-function table loads to a single load of that table."""
    if not int(os.environ.get("KERNEL_ACT_DEDUP", "1")):
        return
    try:
        from concourse.hw_specs import get_activation_tables

        tabs = get_activation_tables(nc.m.arch)
        need = {AF.Exp, AF.Ln, AF.Relu, AF.Copy, AF.Identity}
        combined = next(
            i for i, fs in enumerate(tabs.values()) if need <= fs
        )
    except Exception:
        return
    for blk in nc.m.functions[0].blocks:
        first = True
        keep = []
        for inst in blk.instructions:
            if isinstance(inst, mybir.InstLoadActFuncSet) and inst.sync_info is None:
                if first:
                    inst.act_func_set_id = combined
                    first = False
                    keep.append(inst)
                continue
            keep.append(inst)
        if len(keep) != len(blk.instructions):
            blk.instructions[:] = keep


def _prepare_host_inputs(x, Wq, Wk, Wv, Wp, bp, W1, b1, W2, b2, g1, be1, g2, be2):
    f = np.float32
    x = np.asarray(x, f)
    Wq = np.asarray(Wq, f)
    Wk = np.asarray(Wk, f)
    Wv = np.asarray(Wv, f)
    Wp = np.asarray(Wp, f)
    W1 = np.asarray(W1, f)
    W2 = np.asarray(W2, f)
    bp = np.asarray(bp, f)
    b1 = np.asarray(b1, f)
    b2 = np.asarray(b2, f)
    g1 = np.asarray(g1, f)
    be1 = np.asarray(be1, f)
    g2 = np.asarray(g2, f)
    be2 = np.asarray(be2, f)

    # stack per-head QKV weights: (H, C, DH) -> (C, C) with d = h*DH + dh
    Wq_all = np.transpose(Wq, (1, 0, 2)).reshape(C, C)
    Wk_all = np.transpose(Wk, (1, 0, 2)).reshape(C, C)
    Wv_all = np.transpose(Wv, (1, 0, 2)).reshape(C, C)

    scale = 1.0 / np.sqrt(np.float32(C))
    Wq_eff = (g1[:, None] * Wq_all) * scale
    bq = (be1 @ Wq_all) * scale
    Wk_eff = g1[:, None] * Wk_all
    bk = be1 @ Wk_all
    Wv_eff = g1[:, None] * Wv_all
    bv = be1 @ Wv_all
    W1_eff = g2[:, None] * W1
    b1e = b1 + be2 @ W1

    def chunk_k(w, nk, dt=FP8):  # (K, N) -> (128, nk, N)
        K, N = w.shape
        return np.ascontiguousarray(
            w.reshape(nk, 128, N).transpose(1, 0, 2).astype(dt)
        )

    flags = {
        "bq_nz": bool(np.any(bq != 0)),
        "bk_nz": bool(np.any(bk != 0)),
        "bv_nz": bool(np.any(bv != 0)),
        "bp_nz": bool(np.any(bp != 0)),
        "b1_nz": bool(np.any(b1e != 0)),
        "b2_nz": bool(np.any(b2 != 0)),
    }
    # transposed-score mask: (s0,t0) triu | (s0,t1) ones | (s1,t1) triu
    tri = np.triu(np.ones((128, 128), np.float32))
    cmaskT = np.concatenate([tri, np.ones((128, 128), np.float32), tri], axis=1)

    parts8 = {
        "wq": chunk_k(Wq_eff, NC_),
        "wk": chunk_k(Wk_eff, NC_),
        "wv": chunk_k(Wv_eff, NC_),
        "wp": chunk_k(Wp, NC_),
        "w1": chunk_k(W1_eff, NC_),
        "w2": chunk_k(W2, NF),
    }
    partsbf = {
        "ident": np.eye(128, dtype=np.float32).astype(BF16),
        "cmaskT": np.ascontiguousarray(cmaskT.astype(BF16)),
        "ones64": np.ones((128, 64), dtype=np.float32).astype(BF16),
        # -30 on causally-masked (s > t) positions, added to scores pre-exp
        "trineg": np.ascontiguousarray(
            (-30.0 * (1.0 - tri)).astype(BF16)
        ),
    }
    partsf32 = {
        "bq": np.ascontiguousarray(bq.reshape(NC_, 128).T.astype(f)),
        "bk": np.ascontiguousarray(bk.reshape(NC_, 128).T.astype(f)),
        "b1e": np.ascontiguousarray(b1e.reshape(NF, 128).T.astype(f)),
    }

    def pack(parts, segs, total, dt):
        blob = np.zeros((128, total), dtype=dt)
        for name, (off, dims) in segs.items():
            n = int(np.prod(dims))
            blob[:, off : off + n] = parts[name].reshape(128, n)
        return blob

    blob8 = pack(parts8, F8_SEGS, F8_TOT, FP8)
    blobbf = pack(partsbf, BF_SEGS, BF_TOT, BF16)
    common = {
        "wfp8": np.ascontiguousarray(
            np.concatenate([blob8, blobbf.view(FP8)], axis=1)
        ),
    }
    if flags["bq_nz"] or flags["bk_nz"] or flags["b1_nz"]:
        common["wf32"] = pack(partsf32, F32_SEGS, F32_TOT, f)
    if flags["bv_nz"]:
        common["bv_bc"] = np.ascontiguousarray(np.broadcast_to(bv, (128, C)).astype(f))
    if flags["bp_nz"]:
        common["bp_bc"] = np.ascontiguousarray(np.broadcast_to(bp, (128, C)).astype(f))
    if flags["b2_nz"]:
        common["b2_bc"] = np.ascontiguousarray(np.broadcast_to(b2, (128, C)).astype(f))
    return x, common, flags


def kernel(x, Wq, Wk, Wv, Wp, bp, W1, b1, W2, b2, g1, be1, g2, be2):
    global LAST_EXEC_NS, LAST_RESULTS
    x, common, flags = _prepare_host_inputs(
        x, Wq, Wk, Wv, Wp, bp, W1, b1, W2, b2, g1, be1, g2, be2
    )
    key = tuple(sorted(flags.items()))
    if key not in _PROGRAM_CACHE:
        _PROGRAM_CACHE[key] = _build_program(flags)
    nc = _PROGRAM_CACHE[key]

    in_maps = []
    for c in range(N_CORES):
        m = dict(common)
        m["x_shard"] = np.ascontiguousarray(x[c * BPC : (c + 1) * BPC])
        in_maps.append(m)

    kwargs = {}
    if PROFILE:
        kwargs["trace"] = True
        if TRACE_DIR:
            kwargs["tmpdir"] = TRACE_DIR
    res = run_bass_kernel_spmd(nc, in_maps, core_ids=list(range(N_CORES)), **kwargs)
    LAST_EXEC_NS = res.exec_time_ns
    LAST_RESULTS = res
    out = np.concatenate([np.asarray(r["out"]) for r in res.results], axis=0)
    return out.astype(np.float32)

